# revision 2
# baseline (speedup 1.0000x reference)
"""MixedDecoder (moe_routing) Trainium2 Bass kernel.

Data-parallel over batch: B=1024 split as 128 samples per core across 8
NeuronCores.  Each core runs the full gate MLP + 3 mixed-expert layers on
its batch slice with all expert weights streamed from HBM.

Math per layer:  out = sum_e coeff[:,e] * (inp @ w[e]) + coeff @ b
implemented as PE-only accumulation: the coeff scaling is applied to the
K-transposed input tiles (one DVE multiply per (expert, k-tile)), so every
(expert, k-tile) matmul plus the mixed-bias matmul accumulates into one
PSUM bank.
"""

import numpy as np
import sys

sys.path.insert(0, "/opt/trn_rl_repo")

import concourse.bass as bass
import concourse.mybir as mybir
import concourse.tile as tile
from concourse.masks import make_identity

F32 = mybir.dt.float32
# Matmul operand dtype: float32 (4 cyc/row) or float32r (1 cyc/row, N>=256).
MM_DT = mybir.dt.float32r
AF = mybir.ActivationFunctionType
ALU = mybir.AluOpType

B, LAT, FCON = 1024, 64, 256
IN_SZ = LAT + FCON              # 320
HID, E, GATE_H = 512, 8, 64
INTER = LAT + HID               # 576
OUT_SZ = 512
NCORES = 8
BL = B // NCORES                # 128

LAST_EXEC_NS = None
LAST_RESULTS = None


def _split_multi_waits(bir_str):
    """Walrus in this toolchain accepts at most one sync wait per
    instruction.  Hoist extra on_wait entries onto standalone
    EventSemaphore instructions inserted just before, on the same engine
    queue (engine program order == list order)."""
    import json

    d = json.loads(bir_str)
    ctr = [0]

    def fix_list(lst):
        out = []
        for ins in lst:
            if isinstance(ins, dict) and "opcode" in ins and "sync_info" in ins:
                si = ins.get("sync_info") or {}
                ow = si.get("on_wait") or []
                if len(ow) > 1:
                    for w in ow[:-1]:
                        ctr[0] += 1
                        out.append({
                            "debug": ins.get("debug", 0),
                            "engine": ins["engine"],
                            "ins": [], "outs": [],
                            "name": f"splitwait_{ctr[0]}",
                            "opcode": "EventSemaphore",
                            "sync_info": {"on_update": [], "on_wait": [w]},
                        })
                    si["on_wait"] = [ow[-1]]
            out.append(ins)
        return out

    def walk(o):
        if isinstance(o, dict):
            for k, v in o.items():
                if (isinstance(v, list) and v and isinstance(v[0], dict)
                        and "opcode" in v[0]):
                    o[k] = fix_list(v)
                    for ins in o[k]:
                        walk(ins)
                else:
                    walk(v)
        elif isinstance(o, list):
            for v in o:
                walk(v)

    walk(d)
    return json.dumps(d).encode(), ctr[0]


def _install_wait_splitter():
    from concourse import bass2jax, bass_utils

    orig = bass_utils.compile_bir_kernel

    def wrapper(bir_str, *a, **k):
        if isinstance(bir_str, str):
            bir_str = bir_str.encode()
        new, n = _split_multi_waits(bir_str)
        return orig(new, *a, **k)

    bass2jax.compile_bir_kernel = wrapper
    return orig


def _elu(nc, pool, psum_ap, out_tile, P, N):
    """out = elu(psum):  relu(x) - relu(1 - exp(x))."""
    e = pool.tile([P, N], F32, tag="elu_e")
    r = pool.tile([P, N], F32, tag="elu_r")
    s = pool.tile([P, N], F32, tag="elu_s")
    nc.scalar.activation(e[:], psum_ap, AF.Exp)
    nc.scalar.activation(r[:], psum_ap, AF.Relu)
    nc.scalar.activation(s[:], e[:], AF.Relu, bias=1.0, scale=-1.0)
    nc.vector.tensor_tensor(out=out_tile, in0=r[:], in1=s[:], op=ALU.subtract)


def build_program():
    nc = bass.Bass()

    xta_d = nc.declare_dram_parameter("xta", [IN_SZ + 1, BL], F32, isOutput=False)
    w0_d = nc.declare_dram_parameter("w0", [E, IN_SZ, HID], MM_DT, isOutput=False)
    w1_d = nc.declare_dram_parameter("w1", [E, INTER, HID], MM_DT, isOutput=False)
    w2_d = nc.declare_dram_parameter("w2", [E, INTER, OUT_SZ], MM_DT, isOutput=False)
    b0_d = nc.declare_dram_parameter("b0", [E, HID], F32, isOutput=False)
    b1_d = nc.declare_dram_parameter("b1", [E, HID], F32, isOutput=False)
    b2_d = nc.declare_dram_parameter("b2", [E, OUT_SZ], F32, isOutput=False)
    gw1_d = nc.declare_dram_parameter("gw1a", [IN_SZ + 1, GATE_H], F32, isOutput=False)
    gw2_d = nc.declare_dram_parameter("gw2a", [GATE_H + 1, GATE_H], F32, isOutput=False)
    gw3_d = nc.declare_dram_parameter("gw3a", [GATE_H + 1, E], F32, isOutput=False)
    oh_d = nc.declare_dram_parameter("oh", [E, E * 128], F32, isOutput=False)
    out_d = nc.declare_dram_parameter("out", [BL, OUT_SZ], F32, isOutput=True)

    with tile.TileContext(nc) as tc:
        with (
            tc.tile_pool(name="const", bufs=1) as cpool,
            tc.tile_pool(name="gate", bufs=1) as gpool,
            tc.tile_pool(name="acts", bufs=1) as apool,
            tc.tile_pool(name="elu", bufs=2) as epool,
            tc.tile_pool(name="wts", bufs=48) as wpool,
            tc.tile_pool(name="scaled", bufs=48) as spool,
            tc.tile_pool(name="biasp", bufs=3) as bpool,
            tc.tile_pool(name="ps_main", bufs=2, space="PSUM") as ps_main,
            tc.tile_pool(name="ps_aux", bufs=2, space="PSUM") as ps_aux,
            tc.tile_pool(name="ps_tr", bufs=1, space="PSUM") as ps_tr_pool,
            tc.tile_pool(name="ps_bc", bufs=1, space="PSUM") as ps_bc,
        ):
            ident = cpool.tile([128, 128], F32)
            make_identity(nc, ident[:])
            ohs = cpool.tile([E, E * 128], F32)
            nc.gpsimd.dma_start(ohs[:], oh_d[:])
            oh = cpool.tile([E, E * 128], F32)
            nc.vector.tensor_copy(oh[:], ohs[:])

            # ---- load input xT (augmented with ones row) : [321, 128]
            xt = []  # k-tiles of x^T: [128],[128],[65(incl ones)]
            for i, (r0, r1) in enumerate([(0, 128), (128, 256), (256, 321)]):
                st = cpool.tile([128, BL], F32, tag=f"xts{i}")
                nc.gpsimd.dma_start(st[0 : r1 - r0, :], xta_d[r0:r1, :])
                t = cpool.tile([128, BL], F32, tag=f"xt{i}")
                nc.vector.tensor_copy(t[0 : r1 - r0, :], st[0 : r1 - r0, :])
                xt.append(t)

            # ---- gate layer 1: h1 = elu(x @ gw1 + gb1)   [128, 64]
            g1w = []
            for i, (r0, r1) in enumerate([(0, 128), (128, 256), (256, 321)]):
                st = gpool.tile([128, GATE_H], F32, tag=f"g1ws{i}")
                nc.gpsimd.dma_start(st[0 : r1 - r0, :], gw1_d[r0:r1, :])
                t = gpool.tile([128, GATE_H], F32, tag=f"g1w{i}")
                nc.vector.tensor_copy(t[0 : r1 - r0, :], st[0 : r1 - r0, :])
                g1w.append(t)
            ps_g = ps_aux.tile([128, GATE_H], F32, tag="g")
            for i, (r0, r1) in enumerate([(0, 128), (128, 256), (256, 321)]):
                k = r1 - r0
                nc.tensor.matmul(
                    ps_g[:], xt[i][0:k, :], g1w[i][0:k, :],
                    start=(i == 0), stop=(i == 2),
                )
            h1 = gpool.tile([128, GATE_H], F32)
            _elu(nc, epool, ps_g[:], h1[:], 128, GATE_H)

            # h1Ta: [65, 128] with ones row at index 64
            ps_t = ps_aux.tile([128, 128], F32, tag="g")
            nc.tensor.transpose(ps_t[0:GATE_H, :], h1[:], ident[:])
            h1ta = gpool.tile([GATE_H + 1, 128], F32, tag="h1ta")
            nc.vector.tensor_copy(h1ta[0:GATE_H, :], ps_t[0:GATE_H, :])
            nc.gpsimd.memset(h1ta[GATE_H : GATE_H + 1, :], 1.0)

            # ---- gate layer 2
            g2ws = gpool.tile([GATE_H + 1, GATE_H], F32)
            nc.gpsimd.dma_start(g2ws[:], gw2_d[:])
            g2w = gpool.tile([GATE_H + 1, GATE_H], F32)
            nc.vector.tensor_copy(g2w[:], g2ws[:])
            ps_g2 = ps_aux.tile([128, GATE_H], F32, tag="g")
            nc.tensor.matmul(ps_g2[:], h1ta[:], g2w[:], start=True, stop=True)
            h2 = gpool.tile([128, GATE_H], F32)
            _elu(nc, epool, ps_g2[:], h2[:], 128, GATE_H)

            ps_t2 = ps_aux.tile([128, 128], F32, tag="g")
            nc.tensor.transpose(ps_t2[0:GATE_H, :], h2[:], ident[:])
            h2ta = gpool.tile([GATE_H + 1, 128], F32, tag="h2ta")
            nc.vector.tensor_copy(h2ta[0:GATE_H, :], ps_t2[0:GATE_H, :])
            nc.gpsimd.memset(h2ta[GATE_H : GATE_H + 1, :], 1.0)

            # ---- gate layer 3 + softmax -> coeff [128, 8]
            g3ws = gpool.tile([GATE_H + 1, E], F32)
            nc.gpsimd.dma_start(g3ws[:], gw3_d[:])
            g3w = gpool.tile([GATE_H + 1, E], F32)
            nc.vector.tensor_copy(g3w[:], g3ws[:])
            ps_g3 = ps_aux.tile([128, E], F32, tag="g")
            nc.tensor.matmul(ps_g3[:], h2ta[:], g3w[:], start=True, stop=True)

            negmax = gpool.tile([128, 1], F32)
            nc.vector.tensor_reduce(
                negmax[:], ps_g3[:], axis=mybir.AxisListType.X, op=ALU.max,
                negate=True,
            )
            esum = gpool.tile([128, 1], F32)
            enum = gpool.tile([128, E], F32)
            nc.scalar.activation(
                enum[:], ps_g3[:], AF.Exp, bias=negmax[:], scale=1.0,
                accum_out=esum[:],
            )
            rec = gpool.tile([128, 1], F32)
            nc.vector.reciprocal(rec[:], esum[:])
            coeff = gpool.tile([128, E], F32)
            nc.vector.tensor_scalar_mul(coeff[:], enum[:], rec[:])

            # coeff^T [8, 128]
            ps_ct = ps_aux.tile([128, 128], F32, tag="g")
            nc.tensor.transpose(ps_ct[0:E, :], coeff[:], ident[:])
            ct = gpool.tile([E, 128], F32)
            nc.vector.tensor_copy(ct[:], ps_ct[0:E, :])

            # broadcast coeff columns: bc[e][p, b] = coeff[b, e]
            bc_sb = []
            for g in range(2):
                ps_b = ps_bc.tile([128, 512], F32)
                for j in range(4):
                    e = g * 4 + j
                    nc.tensor.matmul(
                        ps_b[:, j * 128 : (j + 1) * 128],
                        oh[:, e * 128 : (e + 1) * 128], ct[:],
                        start=True, stop=True,
                    )
                sb = apool.tile([128, 512], F32, tag=f"bc{g}")
                nc.vector.tensor_copy(sb[:], ps_b[:])
                bc_sb.append(sb)

            def bc_e(e):
                return bc_sb[e // 4][:, (e % 4) * 128 : (e % 4 + 1) * 128]

            # ---- 3 mixed-expert layers
            layer_cfg = [
                (w0_d, b0_d, [(0, 128), (128, 256), (256, 320)], HID, True),
                (w1_d, b1_d, [(0, 64), (64, 192), (192, 320), (320, 448), (448, 576)], HID, True),
                (w2_d, b2_d, [(0, 64), (64, 192), (192, 320), (320, 448), (448, 576)], OUT_SZ, False),
            ]

            # k-tile source APs for layer 0 (x^T) vs layers 1/2 (z^T + h^T)
            hT = None
            for li, (w_d, b_d, kt, NOUT, has_act) in enumerate(layer_cfg):
                if li == 0:
                    def src(i, k):
                        return xt[i][0:k, :]
                else:
                    hT_cur = hT

                    def src(i, k, hT_cur=hT_cur):
                        if i == 0:
                            return xt[0][0:64, :]  # z^T
                        return hT_cur[:, (i - 1) * 128 : i * 128]

                biass = bpool.tile([E, NOUT], F32, tag="biass")
                nc.sync.dma_start(biass[:], b_d[:])
                bias = bpool.tile([E, NOUT], F32, tag="bias")
                nc.vector.tensor_copy(bias[:], biass[:])

                ps_o = ps_main.tile([128, NOUT], F32)
                first = True
                for e in range(E):
                    for i, (r0, r1) in enumerate(kt):
                        k = r1 - r0
                        wt = wpool.tile([128, NOUT], MM_DT, tag="w")
                        nc.sync.dma_start(wt[0:k, :], w_d[e, r0:r1, :])
                        a = spool.tile([128, 128], MM_DT, tag="a")
                        nc.vector.tensor_tensor(
                            out=a[0:k, :], in0=src(i, k), in1=bc_e(e)[0:k, :],
                            op=ALU.mult,
                        )
                        nc.tensor.matmul(
                            ps_o[:], a[0:k, :], wt[0:k, :],
                            start=first, stop=False,
                        )
                        first = False
                # mixed bias: coeff @ b  ==  (ct).T @ b
                nc.tensor.matmul(ps_o[:], ct[:], bias[:], start=False, stop=True)

                if has_act:
                    h = apool.tile([128, NOUT], F32, tag=f"h{li}")
                    _elu(nc, epool, ps_o[:], h[:], 128, NOUT)
                    # transpose h -> hT [128, 4*128] (slice t = rows t*128..)
                    ps_tr = ps_tr_pool.tile([128, 512], F32, tag="ps_tr_h")
                    for t in range(4):
                        nc.tensor.transpose(
                            ps_tr[:, t * 128 : (t + 1) * 128],
                            h[:, t * 128 : (t + 1) * 128],
                            ident[:],
                        )
                    hT = apool.tile([128, 512], F32, tag=f"hT{li}")
                    nc.vector.tensor_copy(hT[:], ps_tr[:])
                else:
                    res = apool.tile([128, NOUT], F32, tag="res")
                    nc.vector.tensor_copy(res[:], ps_o[:])
                    nc.sync.dma_start(out_d[:], res[:])

    return nc


def _one_hot_expand():
    oh = np.zeros((E, E * 128), np.float32)
    for e in range(E):
        oh[e, e * 128 : (e + 1) * 128] = 1.0
    return oh


def prepare(z, c, w0, b0, w1, b1, w2, b2, gw1, gb1, gw2, gb2, gw3, gb3):
    """Build the Bass program and per-core input maps."""
    _install_wait_splitter()

    f = np.float32
    x = np.concatenate([z, c], axis=1).astype(f)            # [B, 320]
    xta = np.concatenate([x.T, np.ones((1, B), f)], axis=0)  # [321, B]
    gw1a = np.concatenate([gw1, gb1[None, :]], axis=0).astype(f)
    gw2a = np.concatenate([gw2, gb2[None, :]], axis=0).astype(f)
    gw3a = np.concatenate([gw3, gb3[None, :]], axis=0).astype(f)
    shared = {
        "w0": np.ascontiguousarray(w0, f), "b0": np.ascontiguousarray(b0, f),
        "w1": np.ascontiguousarray(w1, f), "b1": np.ascontiguousarray(b1, f),
        "w2": np.ascontiguousarray(w2, f), "b2": np.ascontiguousarray(b2, f),
        "gw1a": gw1a, "gw2a": gw2a, "gw3a": gw3a,
        "oh": _one_hot_expand(),
    }
    in_maps = []
    for i in range(NCORES):
        m = dict(shared)
        m["xta"] = np.ascontiguousarray(xta[:, i * BL : (i + 1) * BL])
        in_maps.append(m)

    return build_program(), in_maps


def assemble(results):
    return np.concatenate([results[i]["out"] for i in range(NCORES)], axis=0)


def kernel(**inputs):
    global LAST_EXEC_NS, LAST_RESULTS
    from concourse.bass_utils import run_bass_kernel_spmd

    nc, in_maps = prepare(**inputs)
    r = run_bass_kernel_spmd(nc, in_maps, list(range(NCORES)))
    LAST_EXEC_NS = r.exec_time_ns
    LAST_RESULTS = r
    return assemble(r.results)

